# revision 1
# baseline (speedup 1.0000x reference)
"""Multi-head attention (B=4, Q=K=2048, N=12 heads, H=64) on 8 TRN2 NeuronCores.

Sharding: core c handles batch b = c // 2 and head-group g = c % 2 (6 local
heads, output columns [g*384:(g+1)*384]). Pure data-parallel, no collectives.

v3 design:
  - Inputs arrive HOST-TRANSPOSED bf16 (xT [768, 2048]) and weights arrive
    host-packed dt-major, so every DMA is a fat contiguous transfer.
  - Heads are processed in PAIRS sharing a 128-partition m-tile ([128 =
    2 heads x 64 h-dims, seq]).  Score matmuls are ROW-TILED: head A's
    64-row contraction occupies PE rows 0-63 (tile_position (0,0)), head
    B's rows 64-127 (tile_position (64,0)).  The two matmuls execute
    concurrently in the array halves (64-deep reorder window), recovering
    full PE throughput for the H=64 contraction without fp8 DoubleRow's
    doubled LDWEIGHTS cost.  All-bf16 keeps rel err ~4e-3.
  - exp runs on the Act engine (f32 PSUM scores -> bf16 SBUF e tiles) and
    does nothing else; Act is the pacing engine (~200 us busy).
  - PV (context) matmuls are bf16 with the ones-column denominator trick,
    lagging the exp stream by E_LAG chunks so v-projections hide in the
    early exp shadow; the lag shrinks after the crunch to shorten the tail.
  - Per (pair, qb): 8 chunk steps, each = 2 score matmuls per head over 2
    k-tiles -> exp per head -> lagged PV; projections for later pairs and
    the v/output paths are emitted as slot-scheduled fillers.
"""

import sys
from contextlib import ExitStack

sys.path.insert(0, "/opt/trn_rl_repo")

import numpy as np
import ml_dtypes

import concourse.bass as bass
import concourse.tile as tile
from concourse import bacc, mybir
from concourse.bass_utils import run_bass_kernel_spmd

F32 = mybir.dt.float32
F32R = mybir.dt.float32r
BF16 = mybir.dt.bfloat16
EXPF = mybir.ActivationFunctionType.Exp

B, SEQ, N_HEADS, H = 4, 2048, 12, 64
D = N_HEADS * H            # 768
NH = 6                     # heads per core
NM = NH // 2               # head pairs (m-tiles)
DG = NH * H                # 384 output cols per core
P = 128
DT = D // P                # 6 d-tiles
QB = SEQ // 512            # 4 q blocks of 512
CHUNKS = (3, 3, 3, 3, 3, 1)   # k-tiles per chunk step (exp width 3*512)
NCH = len(CHUNKS)
RT = SEQ // P              # 16 k row tiles
E_LAG = 12                 # PV trails exp by this many stream entries
E_LAG_LATE = 4
LAG_SWITCH = 48
SCALE = 0.125              # 1/sqrt(64)

NPBF16 = ml_dtypes.bfloat16


def build_nc(reps: int = 1, diag: str = ""):
    nc = bacc.Bacc("TRN2", target_bir_lowering=False, debug=False, num_devices=8)

    xq_d = nc.dram_tensor("xqT", [D, SEQ], BF16, kind="ExternalInput").ap()
    xk_d = nc.dram_tensor("xkT", [D, SEQ], BF16, kind="ExternalInput").ap()
    xv_d = nc.dram_tensor("xvT", [D, SEQ], BF16, kind="ExternalInput").ap()
    x_d = {"q": xq_d, "k": xk_d, "v": xv_d}
    wq_d = nc.dram_tensor("wq", [P, DT * DG], BF16, kind="ExternalInput").ap()
    wk_d = nc.dram_tensor("wk", [P, DT * DG], BF16, kind="ExternalInput").ap()
    wv_d = nc.dram_tensor("wv", [P, DT * DG], BF16, kind="ExternalInput").ap()
    identf_d = nc.dram_tensor("identf", [P, P], F32R, kind="ExternalInput").ap()
    out_d = nc.dram_tensor("out", [SEQ, DG], F32, kind="ExternalOutput").ap()

    with tile.TileContext(nc) as tc:
     for _rep in range(reps):
      with ExitStack() as stack:
        singles = stack.enter_context(tc.tile_pool(name="singles", bufs=1))
        identf = singles.tile([P, P], F32R)
        w_sb = {}
        for t in ("q", "k", "v"):
            w_sb[t] = singles.tile([P, DT, DG], BF16, tag=f"w{t}", name=f"w{t}")

        xTp = stack.enter_context(tc.tile_pool(name="xT", bufs=1))
        slabs = {t: [xTp.tile([P, SEQ], BF16, tag=f"{t}T{dt}", name=f"{t}T{dt}")
                     for dt in range(DT)] for t in ("k", "q", "v")}

        # projected q/k per m-tile: [128 = pair x 64h, seq] bf16
        qkT = {(t, m): singles.tile([P, SEQ], BF16, tag=f"{t}m{m}",
                                    name=f"{t}m{m}")
               for t in ("q", "k") for m in range(NM)}

        vpool = stack.enter_context(tc.tile_pool(name="v", bufs=1))
        v_sb = [vpool.tile([P, NH, H + 1], BF16, tag=f"v{kt}", name=f"v{kt}")
                for kt in range(RT)]
        for kt in range(RT):
            nc.gpsimd.memset(v_sb[kt][:, :, H:H + 1], 1.0)

        # ---- input loads, single SP queue; FIFO order IS the schedule -----
        def x_load(t):
            for dt in range(DT):
                nc.sync.dma_start(out=slabs[t][dt],
                                  in_=x_d[t][dt * P:(dt + 1) * P, :])

        def w_load(t, wd):
            nc.sync.dma_start(out=w_sb[t].rearrange("p dt c -> p (dt c)"),
                              in_=wd)
        x_load("k")
        w_load("k", wk_d)
        w_load("q", wq_d)
        x_load("q")
        w_load("v", wv_d)
        x_load("v")
        nc.sync.dma_start(out=identf, in_=identf_d)

        # ---- main pools ----------------------------------------------------
        psProj = stack.enter_context(tc.tile_pool(name="psProj", bufs=1,
                                                  space="PSUM"))
        psS = stack.enter_context(tc.tile_pool(name="psS", bufs=1, space="PSUM"))
        psPV = stack.enter_context(tc.tile_pool(name="psPV", bufs=1,
                                                space="PSUM"))
        expp = stack.enter_context(tc.tile_pool(name="expp", bufs=E_LAG + 2))
        cTp = stack.enter_context(tc.tile_pool(name="cT", bufs=2))
        outp = stack.enter_context(tc.tile_pool(name="outp", bufs=1))
        smallp = stack.enter_context(tc.tile_pool(name="small", bufs=4))

        out_tiles = {(qb, st): outp.tile([P, DG], F32, tag=f"o{qb}{st}",
                                         name=f"o{qb}{st}")
                     for qb in range(QB) for st in range(4)}

        # ---- emission helpers ---------------------------------------------
        def proj_m(t, m, ch):
            """Project q/k m-tile chunk: q/k-range [ch*512,(ch+1)*512)."""
            pj = psProj.tile([P, 512], F32, tag="pj", name=f"pj{t}{m}{ch}")
            for dt in range(DT):
                nc.tensor.matmul(
                    pj, w_sb[t][:, dt, m * P:(m + 1) * P],
                    slabs[t][dt][:, ch * 512:(ch + 1) * 512],
                    start=(dt == 0), stop=(dt == DT - 1))
            nc.vector.tensor_copy(
                out=qkT[t, m][:, ch * 512:(ch + 1) * 512], in_=pj)

        def vproj_chunk(kt):
            pj = psProj.tile([P, 512], F32, tag="pj", name=f"pjv{kt}")
            for dt in range(DT):
                nc.tensor.matmul(pj[:, 0:DG],
                                 slabs["v"][dt][:, kt * P:(kt + 1) * P],
                                 w_sb["v"][:, dt, :],
                                 start=(dt == 0), stop=(dt == DT - 1))
            nc.vector.tensor_copy(
                out=v_sb[kt][:, :, 0:H],
                in_=pj[:, 0:DG].rearrange("p (n h) -> p n h", h=H))

        def finish_qb(head, qb, pv):
            cT = cTp.tile([P, 512], F32R, tag="cT", name="cT")
            nc.vector.tensor_copy(out=cT[0:H + 1, :], in_=pv)
            for st in range(4):
                ctp = psProj.tile([P, 512], F32, tag="pj", name="ctp")
                nc.tensor.transpose(ctp[:, 0:P].bitcast(F32R),
                                    cT[:, st * P:(st + 1) * P], identf)
                rec = smallp.tile([P, 1], F32, tag="rec", name="rec")
                nc.vector.reciprocal(rec, ctp[:, H:H + 1])
                nc.vector.tensor_scalar_mul(
                    out_tiles[qb, st][:, head * H:(head + 1) * H],
                    ctp[:, 0:H], rec)
            if head == NH - 1:
                for st in range(4):
                    nc.sync.dma_start(
                        out=out_d[qb * 512 + st * P: qb * 512 + (st + 1) * P, :],
                        in_=out_tiles[qb, st])

        # ---- filler schedule (slot = stream index of the A-head entry) ----
        # stream: (m, qb, c, sub) -> 16 entries per (m, qb); m0 spans
        # entries 0-63, m1 64-127, m2 128-191.
        # deadlines: q m0 qb_i by entry 16i; vp kt by E_LAG + 2*(kt//2);
        # k/q m1 by 64, m2 by 128.
        def pj_item(t, m, ch):
            return lambda: proj_m(t, m, ch)

        def vp(k):
            return lambda: vproj_chunk(k)

        sched = {2: [pj_item("q", 0, 1)], 13: [pj_item("q", 0, 2)],
                 25: [pj_item("q", 0, 3)]}
        for k in range(RT):
            # vp k must land by stream entry 2*(k//3) + E_LAG (PV deadline)
            slot = 4 + k
            assert slot <= 2 * (k // 3) + E_LAG - 1
            sched.setdefault(slot, []).append(vp(k))
        later = ([("k", 1, c) for c in range(4)] +
                 [("q", 1, c) for c in range(4)] +
                 [("k", 2, c) for c in range(4)] +
                 [("q", 2, c) for c in range(4)])
        for i, (t, m, c) in enumerate(later):
            sched.setdefault(26 + 3 * i, []).append(pj_item(t, m, c))
        assert 26 + 3 * 7 < 48 and 26 + 3 * 15 < 96  # m1 by 48, m2 by 96

        # ---- prologue: k m0 fully, then q m0 first block -------------------
        for c in range(4):
            proj_m("k", 0, c)
        proj_m("q", 0, 0)

        # ---- main loop -----------------------------------------------------
        stream = [(m, qb, c, sub) for m in range(NM) for qb in range(QB)
                  for c in range(NCH) for sub in (0, 1)]
        pv_tiles = {}
        e_tiles = {}
        s_pair = [None]

        KT0 = [sum(CHUNKS[:i]) for i in range(NCH)]

        def emit_pv(m, qb, c, sub):
            h = 2 * m + sub
            if (h, qb) not in pv_tiles:
                pv_tiles[h, qb] = psPV.tile([H + 1, 512], F32, tag="pv",
                                            name=f"pv{h}{qb}")
            pv = pv_tiles[h, qb]
            e = e_tiles.pop((h, qb, c))
            for j in range(CHUNKS[c]):
                kt = KT0[c] + j
                nc.tensor.matmul(pv, v_sb[kt][:, h, :],
                                 e[:, j * 512:(j + 1) * 512],
                                 start=(kt == 0), stop=(kt == RT - 1))
            if c == NCH - 1:
                finish_qb(h, qb, pv_tiles.pop((h, qb)))

        pv_next = [0]

        def drain_pv(idx):
            lag = E_LAG if idx < LAG_SWITCH else E_LAG_LATE
            limit = 2 if idx >= LAG_SWITCH else 1
            n = 0
            while pv_next[0] <= idx - lag and n < limit:
                emit_pv(*stream[pv_next[0]])
                pv_next[0] += 1
                n += 1

        for idx, (m, qb, c, sub) in enumerate(stream):
            for item in sched.get(idx, ()):
                item()

            csz = CHUNKS[c]
            if sub == 0:
                # both heads' score matmuls, row-tiled to overlap in the
                # PE array halves: head A rows 0-63, head B rows 64-127
                kTm, qTm = qkT["k", m], qkT["q", m]
                svals = []
                for s_i in (0, 1):
                    svals.append(psS.tile(
                        [P, 1536], F32, tag=("sA" if s_i == 0 else "sB"),
                        name="s"))
                for j in range(csz):
                    kt = KT0[c] + j
                    for s_i in (0, 1):
                        lo = 64 * s_i
                        nc.tensor.matmul(
                            svals[s_i][:, j * 512:(j + 1) * 512],
                            kTm[lo:lo + 64, kt * P:(kt + 1) * P],
                            qTm[lo:lo + 64, qb * 512:(qb + 1) * 512],
                            start=True, stop=True,
                            tile_position=(lo, 0))
                s_pair[0] = svals
            h = 2 * m + sub
            e = expp.tile([P, 1536], BF16, tag="e", name="e")
            nc.scalar.activation(out=e[:, 0:csz * 512],
                                 in_=s_pair[0][sub][:, 0:csz * 512],
                                 func=EXPF, scale=SCALE)
            e_tiles[h, qb, c] = e

            drain_pv(idx)

        while pv_next[0] < len(stream):
            emit_pv(*stream[pv_next[0]])
            pv_next[0] += 1

    nc.compile()
    return nc


_NC_CACHE = {}


def _get_nc(has_bias=False, has_mask=False, reps=1, diag=""):
    assert not has_bias and not has_mask
    key = (reps, diag)
    if key not in _NC_CACHE:
        _NC_CACHE[key] = build_nc(reps, diag)
    return _NC_CACHE[key]


def _host_dtmajor(W):
    """[768, C] -> partition-major [128, 6*C]: row p holds dt-tile rows."""
    C = W.shape[1]
    return np.ascontiguousarray(
        W.reshape(DT, P, C).transpose(1, 0, 2).reshape(P, DT * C))


def shard_inputs(query, key, value, mask, Wq, bq, Wk, bk, Wv, bv,
                 batch_size=B, num_heads=N_HEADS):
    query = np.asarray(query, dtype=np.float32)
    key = np.asarray(key, dtype=np.float32)
    value = np.asarray(value, dtype=np.float32)
    Wq = np.asarray(Wq, dtype=np.float32)
    Wk = np.asarray(Wk, dtype=np.float32)
    Wv = np.asarray(Wv, dtype=np.float32)
    assert query.shape == (B * SEQ, D) and key.shape == (B * SEQ, D)
    assert int(batch_size) == B and int(num_heads) == N_HEADS

    has_bias = bool(np.any(bq) or np.any(bk) or np.any(bv))
    has_mask = bool(np.any(mask))

    qb16 = query.astype(NPBF16)
    kb16 = key.astype(NPBF16)
    vb16 = value.astype(NPBF16)

    in_maps = []
    for c in range(8):
        b, g = divmod(c, 2)
        rows = slice(b * SEQ, (b + 1) * SEQ)
        cols = slice(g * DG, (g + 1) * DG)
        m = {
            "identf": np.eye(P, dtype=np.float32),
            "xqT": np.ascontiguousarray(qb16[rows].T),
            "xkT": np.ascontiguousarray(kb16[rows].T),
            "xvT": np.ascontiguousarray(vb16[rows].T),
            "wq": _host_dtmajor(Wq[:, cols]).astype(NPBF16),
            "wk": _host_dtmajor(Wk[:, cols]).astype(NPBF16),
            "wv": _host_dtmajor(Wv[:, cols]).astype(NPBF16),
        }
        in_maps.append(m)
    return in_maps, has_bias, has_mask


def make_in_maps(inputs):
    return shard_inputs(**{k: inputs[k] for k in
                           ("query", "key", "value", "mask", "Wq", "bq",
                            "Wk", "bk", "Wv", "bv", "batch_size", "num_heads")})[0]


def assemble(results):
    full = np.empty((B * SEQ, D), dtype=np.float32)
    for c in range(8):
        b, g = divmod(c, 2)
        full[b * SEQ:(b + 1) * SEQ, g * DG:(g + 1) * DG] = results[c]["out"]
    return full


def _reference_fallback(query, key, value, mask, Wq, bq, Wk, bk, Wv, bv,
                        batch_size, num_heads):
    b, n = int(batch_size), int(num_heads)
    d = Wq.shape[1]
    h = d // n
    q_len = query.shape[0] // b
    k_len = key.shape[0] // b
    q = (query @ Wq + bq).reshape(b, q_len, n, h).transpose(0, 2, 1, 3)
    k = (key @ Wk + bk).reshape(b, k_len, n, h).transpose(0, 2, 1, 3)
    v = (value @ Wv + bv).reshape(b, k_len, n, h).transpose(0, 2, 1, 3)
    s = np.einsum('bnqh,bnkh->bnqk', q, k) / np.sqrt(h).astype(np.float32)
    s = s + mask
    s = s - s.max(-1, keepdims=True)
    w = np.exp(s)
    w /= w.sum(-1, keepdims=True)
    c = np.einsum('bnqk,bnkh->bqnh', w, v)
    return c.reshape(b * q_len, n * h).astype(np.float32)


def kernel(query, key, value, mask, Wq, bq, Wk, bk, Wv, bv,
           batch_size=B, num_heads=N_HEADS, _trace=False, _trace_kwargs=None):
    in_maps, has_bias, has_mask = shard_inputs(
        query, key, value, mask, Wq, bq, Wk, bk, Wv, bv, batch_size, num_heads)
    if has_bias or has_mask:
        # not exercised by this problem's inputs (zeros); keep a correct path
        return _reference_fallback(query, key, value, mask, Wq, bq, Wk, bk,
                                   Wv, bv, batch_size, num_heads)
    nc = _get_nc()
    res = run_bass_kernel_spmd(nc, in_maps, list(range(8)), trace=_trace,
                               **(_trace_kwargs or {}))
    full = assemble(res.results)
    if _trace:
        return full, res
    return full



# revision 47
# speedup vs baseline: 2.2934x; 2.2934x over previous
"""Multi-head attention (B=4, Q=K=2048, N=12 heads, H=64) on 8 TRN2 NeuronCores.

Sharding: core c handles batch b = c // 2 and head-group g = c % 2 (6 local
heads, output columns [g*384:(g+1)*384]). Pure data-parallel, no collectives.

v7 design (evolved from the 295us v3 baseline via NTFF trace analysis;
measures ~265us with the same tracing methodology that gave v3 328us):
  - Zero-padded per-(head, sub) q/k tiles: every matmul (scores, PV,
    projections) is a plain 128-deep full-array op, so the PE never
    changes tiling mode.  The 64x128 row-tiled score pairs of v3 never
    actually overlapped in production (trace-verified) and the constant
    (64,128)<->(128,128) mode flips cost ~100ns of drain per switch.
  - Split-sub score emission: each head's score matmuls are WAR-gated only
    on that head's own previous exp read (one sub-step earlier), so they
    run inside the other head's exp window and the Act engine (exp) never
    stalls on PE.
  - Wide-ones PV: v tiles are [128 = 64 v-dims | 64 ones] per head, so
    PSUM rows 64-127 of the PV accumulator hold broadcast copies of the
    softmax denominator.  The finish is: fast DVE copies (release the PSUM
    bank), one partition-stacked [128,512] reciprocal per head-pair in
    [128,128] quarters (DVE reciprocal costs ~6.3 cyc per FREE element,
    so partition-stacking both heads halves it), a DVE multiply, and a DMA
    to a TRANSPOSED DRAM output [384, 2048] (host assembles with .T).
    No PE transposes at all.  Finish work is drip-fed (<=2 DVE ops per
    stream entry) so it never delays the projection casts that gate PE.
  - 8 chunks of 2 k-tiles ([128,1024] score tiles = 2 PSUM banks) leave
    4 banks for DOUBLE-buffered projection and PV accumulators -- worth
    ~30us: with single buffers the Tile scheduler serializes the next
    (head, qb) PV/projection group behind the previous group's PSUM reads.
  - Inputs arrive host-packed chunk-major bf16 (6KB contiguous per
    partition per DMA) with weights dt-major; the prologue projects only
    what the first score chunk needs.
"""

import sys
from contextlib import ExitStack

sys.path.insert(0, "/opt/trn_rl_repo")

import numpy as np
import ml_dtypes

import concourse.bass as bass
import concourse.tile as tile
from concourse import bacc, mybir
from concourse.bass_utils import run_bass_kernel_spmd

F32 = mybir.dt.float32
BF16 = mybir.dt.bfloat16
EXPF = mybir.ActivationFunctionType.Exp
MUL = mybir.AluOpType.mult
DIV = mybir.AluOpType.divide

B, SEQ, N_HEADS, H = 4, 2048, 12, 64
D = N_HEADS * H            # 768
NH = 6                     # heads per core
NM = NH // 2               # head pairs (m-tiles)
DG = NH * H                # 384 output cols per core
P = 128
DT = D // P                # 6 d-tiles
QB = SEQ // 512            # 4 q blocks of 512
CHUNKS = (2, 2, 2, 2, 2, 2, 2, 2)   # k-tiles per chunk (exp width 1024)
NCH = len(CHUNKS)
RT = SEQ // P              # 16 k row tiles
E_LAG = 10                 # PV trails exp by this many stream entries
E_LAG_LATE = 3
LAG_SWITCH = 64
SCALE = 0.125              # 1/sqrt(64)
USE_DIVIDE = False         # DVE divide rejected by BIR verifier; recip+mul

NPBF16 = ml_dtypes.bfloat16


def build_nc(reps: int = 1, diag: str = ""):
    nc = bacc.Bacc("TRN2", target_bir_lowering=False, debug=False, num_devices=8)

    # chunk-major packed inputs: [ch][p][dt*512] -> 6KB contiguous per
    # partition per chunk, one fat DMA per (tensor, chunk)
    xq_d = nc.dram_tensor("xqT", [QB, P, DT * 512], BF16,
                          kind="ExternalInput").ap()
    xk_d = nc.dram_tensor("xkT", [QB, P, DT * 512], BF16,
                          kind="ExternalInput").ap()
    xv_d = nc.dram_tensor("xvT", [QB, P, DT * 512], BF16,
                          kind="ExternalInput").ap()
    x_d = {"q": xq_d, "k": xk_d, "v": xv_d}
    wq_d = nc.dram_tensor("wq", [P, DT * DG], BF16, kind="ExternalInput").ap()
    wk_d = nc.dram_tensor("wk", [P, DT * DG], BF16, kind="ExternalInput").ap()
    wv_d = nc.dram_tensor("wv", [P, DT * DG], BF16, kind="ExternalInput").ap()
    out_d = nc.dram_tensor("out", [DG, SEQ], F32, kind="ExternalOutput").ap()

    with tile.TileContext(nc) as tc:
     for _rep in range(reps):
      with ExitStack() as stack:
        singles = stack.enter_context(tc.tile_pool(name="singles", bufs=1))
        w_sb = {}
        for t in ("q", "k", "v"):
            w_sb[t] = singles.tile([P, DT, DG], BF16, tag=f"w{t}", name=f"w{t}")

        xTp = stack.enter_context(tc.tile_pool(name="xT", bufs=1))
        # per (tensor, chunk): [128, dt, 512] bf16
        xch = {(t, ch): xTp.tile([P, DT, 512], BF16, tag=f"{t}C{ch}",
                                 name=f"{t}C{ch}")
               for t in ("k", "q", "v") for ch in range(QB)}

        # projected q/k per (m-tile, sub-head): [128, seq] bf16 with the
        # OTHER head's 64 partitions zeroed.  Score matmuls are then plain
        # 128-deep full-array ops (zero rows contribute nothing), so the PE
        # never changes tiling mode -- no drain penalties, no tile_position.
        qkp = {(t, m, s): singles.tile([P, SEQ], BF16, tag=f"{t}m{m}s{s}",
                                       name=f"{t}m{m}s{s}")
               for t in ("q", "k") for m in range(NM) for s in (0, 1)}

        # v with wide-ones: [:, h, 0:64] = projected v, [:, h, 64:128] = 1.0
        vpool = stack.enter_context(tc.tile_pool(name="v", bufs=1))
        v_sb = [vpool.tile([P, NH, P], BF16, tag=f"v{kt}", name=f"v{kt}")
                for kt in range(RT)]
        for kt in range(RT):
            nc.gpsimd.memset(v_sb[kt][:, :, H:P], 1.0)

        # ---- input loads: k/w on the SP queue, q/v on the gpsimd (SWDGE)
        # queue so the two streams transfer in parallel ------------------
        def x_load_chunk(t, ch, eng):
            eng.dma_start(
                out=xch[t, ch].rearrange("p dt c -> p (dt c)"),
                in_=x_d[t][ch])

        def w_load(t, wd, m, eng):
            # m-tile slice: cols [m*128, (m+1)*128) of each dt block
            eng.dma_start(
                out=w_sb[t][:, :, m * P:(m + 1) * P],
                in_=wd.rearrange("p (dt c) -> p dt c", dt=DT)[:, :,
                                                             m * P:(m + 1) * P])
        w_load("k", wk_d, 0, nc.sync)
        x_load_chunk("k", 0, nc.sync)
        w_load("q", wq_d, 0, nc.sync)
        x_load_chunk("q", 0, nc.sync)
        for ch in range(1, 4):
            x_load_chunk("k", ch, nc.sync)
            x_load_chunk("q", ch, nc.sync)
        nc.sync.dma_start(out=w_sb["v"].rearrange("p dt c -> p (dt c)"),
                          in_=wv_d)
        for ch in range(4):
            x_load_chunk("v", ch, nc.sync)
        for m in range(1, NM):
            w_load("k", wk_d, m, nc.sync)
            w_load("q", wq_d, m, nc.sync)
        # zero the unused half of each per-sub q/k tile (after the DMA
        # triggers so the transfers start immediately; m0 first -- the
        # first scores need those)
        for m in range(NM):
            for t in ("k", "q"):
                nc.gpsimd.memset(qkp[t, m, 0][64:P, :], 0.0)
                nc.gpsimd.memset(qkp[t, m, 1][0:64, :], 0.0)

        # ---- main pools ----------------------------------------------------
        psProj = stack.enter_context(tc.tile_pool(name="psProj", bufs=1,
                                                  space="PSUM"))
        psS = stack.enter_context(tc.tile_pool(name="psS", bufs=1, space="PSUM"))
        psPV = stack.enter_context(tc.tile_pool(name="psPV", bufs=1,
                                                space="PSUM"))
        expp = stack.enter_context(tc.tile_pool(name="expp", bufs=E_LAG + 4))
        outp = stack.enter_context(tc.tile_pool(name="outp", bufs=3))
        smallp = stack.enter_context(tc.tile_pool(name="small", bufs=4))

        # ---- emission helpers ---------------------------------------------
        def proj_m(t, m, ch):
            """Project q/k m-tile chunk: q/k-range [ch*512,(ch+1)*512)."""
            pj = psProj.tile([P, 512], F32, tag="pj", name=f"pj{t}{m}{ch}", bufs=2)
            for dt in range(DT):
                nc.tensor.matmul(
                    pj, w_sb[t][:, dt, m * P:(m + 1) * P],
                    xch[t, ch][:, dt, :],
                    start=(dt == 0), stop=(dt == DT - 1))
            cs = slice(ch * 512, (ch + 1) * 512)
            nc.vector.tensor_copy(out=qkp[t, m, 0][0:64, cs], in_=pj[0:64, :])
            nc.vector.tensor_copy(out=qkp[t, m, 1][64:P, cs], in_=pj[64:P, :])

        def vproj_chunk(kt):
            pj = psProj.tile([P, 512], F32, tag="pj", name=f"pjv{kt}", bufs=2)
            ch, kk = divmod(kt, 4)
            for dt in range(DT):
                nc.tensor.matmul(pj[:, 0:DG],
                                 xch["v", ch][:, dt, kk * P:(kk + 1) * P],
                                 w_sb["v"][:, dt, :],
                                 start=(dt == 0), stop=(dt == DT - 1))
            nc.vector.tensor_copy(
                out=v_sb[kt][:, :, 0:H],
                in_=pj[:, 0:DG].rearrange("p (n h) -> p n h", h=H))

        # finish work is deferred and drip-fed (<=2 DVE ops per stream
        # entry) so its ~5us DVE burst never delays the projection casts
        # that gate PE work -- a burst at each (m,qb) boundary was stalling
        # the PE ~2.5us and re-throttling the HAM clock.
        dve_q = []

        def finish_pair(m, qb, pvA, pvB):
            """pv [128,512]: rows 0-63 context, 64-127 denominator copies.

            Fast DVE copies release the PSUM pv slots (keeps PE fed and the
            HAM clock warm).  The reciprocal costs ~6.3 DVE cycles per FREE
            element regardless of partitions, so both heads' denominators
            are stacked into one [128,512] tile and the reciprocal runs in
            [128,128] quarters to keep DVE queue blockages short.
            """
            hA, hB = 2 * m, 2 * m + 1
            num = outp.tile([P, 512], F32, tag="num", name=f"n{m}{qb}",
                            bufs=2)
            den = smallp.tile([P, 512], F32, tag="den", name=f"d{m}{qb}",
                              bufs=1)
            for pv, lo in ((pvA, 0), (pvB, H)):
                nc.vector.tensor_copy(out=num[lo:lo + H, :], in_=pv[0:H, :])
                nc.vector.tensor_copy(out=den[lo:lo + H, :], in_=pv[H:P, :])
            rcp = smallp.tile([P, 512], F32, tag="rcp", name="rcp", bufs=1)

            def rq(qq):
                s = slice(qq * P, (qq + 1) * P)
                return lambda: nc.vector.reciprocal(rcp[:, s], den[:, s])

            def mq(h, lo, osb):
                def go():
                    nc.vector.tensor_tensor(out=osb[lo:lo + H, :],
                                            in0=num[lo:lo + H, :],
                                            in1=rcp[lo:lo + H, :], op=MUL)
                    nc.sync.dma_start(
                        out=out_d[h * H:(h + 1) * H,
                                  qb * 512:(qb + 1) * 512],
                        in_=osb[lo:lo + H, :])
                return go

            osb = outp.tile([P, 512], F32, tag="osb", name=f"o{m}{qb}",
                            bufs=2)
            ops = [rq(qq) for qq in range(4)] + [mq(hA, 0, osb),
                                                mq(hB, H, osb)]
            if qb == QB - 1 and m >= NM - 2:
                for op in ops:     # last pairs: straight-line, no deferral
                    op()
            else:
                dve_q.extend(ops)

        # ---- filler schedule (slot = stream entry index) -------------------
        # stream: (m, qb, c, sub) -> 12 entries per (m, qb); m0 spans
        # entries 0-47, m1 48-95, m2 96-143.
        def pj_item(t, m, ch):
            return lambda: proj_m(t, m, ch)

        def vp(k):
            return lambda: vproj_chunk(k)

        sched = {2: [pj_item("q", 0, 1)], 13: [pj_item("q", 0, 2)],
                 25: [pj_item("q", 0, 3)]}
        for k in range(RT):
            # vp k must land by stream entry 2*(k//3) + E_LAG (PV deadline;
            # sched items run before drain_pv within an entry)
            slot = 4 + k
            assert slot <= 2 * (k // 2) + E_LAG
            sched.setdefault(slot, []).append(vp(k))
        later = ([("k", 1, c) for c in range(4)] +
                 [("q", 1, c) for c in range(4)] +
                 [("k", 2, c) for c in range(4)] +
                 [("q", 2, c) for c in range(4)])
        for i, (t, m, c) in enumerate(later):
            sched.setdefault(26 + 3 * i, []).append(pj_item(t, m, c))
        assert 26 + 3 * 7 < 64 and 26 + 3 * 15 < 128  # m1 by 64, m2 by 128

        # ---- prologue: only what chunk-0 scores need; rest via sched ------
        proj_m("k", 0, 0)
        proj_m("q", 0, 0)
        # k m0 chunk ch feeds score chunks with k-tiles in [4ch, 4ch+4):
        # score chunk c uses kt 3c..3c+2 -> k-chunk 1 by entry 2, 2 by 4,
        # 3 by 8 (q m0 later chunks already in sched at 2/13/25)
        sched.setdefault(1, []).insert(0, pj_item("k", 0, 1))
        sched.setdefault(3, []).insert(0, pj_item("k", 0, 2))
        sched.setdefault(6, []).insert(0, pj_item("k", 0, 3))

        # ---- main loop -----------------------------------------------------
        stream = [(m, qb, c, sub) for m in range(NM) for qb in range(QB)
                  for c in range(NCH) for sub in (0, 1)]
        pv_tiles = {}
        e_tiles = {}

        KT0 = [sum(CHUNKS[:i]) for i in range(NCH)]

        def emit_pv(m, qb, c, sub):
            h = 2 * m + sub
            if (h, qb) not in pv_tiles:
                pv_tiles[h, qb] = psPV.tile([P, 512], F32, tag="pv",
                                            name=f"pv{h}{qb}", bufs=2)
            pv = pv_tiles[h, qb]
            e = e_tiles.pop((h, qb, c))
            for j in range(CHUNKS[c]):
                kt = KT0[c] + j
                nc.tensor.matmul(pv, v_sb[kt][:, h, :],
                                 e[:, j * 512:(j + 1) * 512],
                                 start=(kt == 0), stop=(kt == RT - 1))
            if c == NCH - 1 and sub == 1:
                finish_pair(m, qb, pv_tiles.pop((2 * m, qb)),
                            pv_tiles.pop((2 * m + 1, qb)))

        pv_next = [0]

        def drain_pv(idx):
            lag = E_LAG if idx < LAG_SWITCH else E_LAG_LATE
            limit = 3
            n = 0
            while pv_next[0] <= idx - lag and n < limit:
                emit_pv(*stream[pv_next[0]])
                pv_next[0] += 1
                n += 1

        # Each sub-head's score matmuls for chunk c are emitted at its own
        # sub step, WAR-gated only on THIS head's previous exp read (which
        # completed one sub-step earlier), so they run inside the other
        # head's exp window and the Act engine never stalls.  Zero-padded
        # qkp tiles make these plain 128-deep matmuls: no mode switches.
        for idx, (m, qb, c, sub) in enumerate(stream):
            csz = CHUNKS[c]
            h = 2 * m + sub
            sval = psS.tile([P, 1024], F32, tag=("sA" if sub == 0 else "sB"),
                            name=f"s{h}{qb}{c}")
            kTm, qTm = qkp["k", m, sub], qkp["q", m, sub]
            for j in range(csz):
                kt = KT0[c] + j
                nc.tensor.matmul(
                    sval[:, j * 512:(j + 1) * 512],
                    kTm[:, kt * P:(kt + 1) * P],
                    qTm[:, qb * 512:(qb + 1) * 512],
                    start=True, stop=True)
            e = expp.tile([P, 1024], BF16, tag="e", name="e")
            nc.scalar.activation(out=e[:, 0:csz * 512],
                                 in_=sval[:, 0:csz * 512],
                                 func=EXPF, scale=SCALE)
            e_tiles[h, qb, c] = e

            for item in sched.get(idx, ()):
                item()
            drain_pv(idx)
            for _ in range(6 if idx >= 168 else 2):
                if dve_q:
                    dve_q.pop(0)()

        while pv_next[0] < len(stream):
            emit_pv(*stream[pv_next[0]])
            pv_next[0] += 1
        while dve_q:
            dve_q.pop(0)()

    nc.compile()
    return nc


_NC_CACHE = {}


def _get_nc(has_bias=False, has_mask=False, reps=1, diag=""):
    assert not has_bias and not has_mask
    key = (reps, diag)
    if key not in _NC_CACHE:
        _NC_CACHE[key] = build_nc(reps, diag)
    return _NC_CACHE[key]


def _host_dtmajor(W):
    """[768, C] -> partition-major [128, 6*C]: row p holds dt-tile rows."""
    C = W.shape[1]
    return np.ascontiguousarray(
        W.reshape(DT, P, C).transpose(1, 0, 2).reshape(P, DT * C))


def _host_chunkmajor(xb16_rows):
    """x [SEQ, D] bf16 -> [QB, P, DT*512]: chunk-major, 6KB/partition DMAs."""
    xT = xb16_rows.T  # [768, 2048]
    return np.ascontiguousarray(
        xT.reshape(DT, P, QB, 512).transpose(2, 1, 0, 3).reshape(
            QB, P, DT * 512))


def shard_inputs(query, key, value, mask, Wq, bq, Wk, bk, Wv, bv,
                 batch_size=B, num_heads=N_HEADS):
    query = np.asarray(query, dtype=np.float32)
    key = np.asarray(key, dtype=np.float32)
    value = np.asarray(value, dtype=np.float32)
    Wq = np.asarray(Wq, dtype=np.float32)
    Wk = np.asarray(Wk, dtype=np.float32)
    Wv = np.asarray(Wv, dtype=np.float32)
    assert query.shape == (B * SEQ, D) and key.shape == (B * SEQ, D)
    assert int(batch_size) == B and int(num_heads) == N_HEADS

    has_bias = bool(np.any(bq) or np.any(bk) or np.any(bv))
    has_mask = bool(np.any(mask))

    qb16 = query.astype(NPBF16)
    kb16 = key.astype(NPBF16)
    vb16 = value.astype(NPBF16)

    in_maps = []
    for c in range(8):
        b, g = divmod(c, 2)
        rows = slice(b * SEQ, (b + 1) * SEQ)
        cols = slice(g * DG, (g + 1) * DG)
        m = {
            "xqT": _host_chunkmajor(qb16[rows]),
            "xkT": _host_chunkmajor(kb16[rows]),
            "xvT": _host_chunkmajor(vb16[rows]),
            "wq": _host_dtmajor(Wq[:, cols]).astype(NPBF16),
            "wk": _host_dtmajor(Wk[:, cols]).astype(NPBF16),
            "wv": _host_dtmajor(Wv[:, cols]).astype(NPBF16),
        }
        in_maps.append(m)
    return in_maps, has_bias, has_mask


def make_in_maps(inputs):
    return shard_inputs(**{k: inputs[k] for k in
                           ("query", "key", "value", "mask", "Wq", "bq",
                            "Wk", "bk", "Wv", "bv", "batch_size", "num_heads")})[0]


def assemble(results):
    full = np.empty((B * SEQ, D), dtype=np.float32)
    for c in range(8):
        b, g = divmod(c, 2)
        full[b * SEQ:(b + 1) * SEQ, g * DG:(g + 1) * DG] = results[c]["out"].T
    return full


def _reference_fallback(query, key, value, mask, Wq, bq, Wk, bk, Wv, bv,
                        batch_size, num_heads):
    b, n = int(batch_size), int(num_heads)
    d = Wq.shape[1]
    h = d // n
    q_len = query.shape[0] // b
    k_len = key.shape[0] // b
    q = (query @ Wq + bq).reshape(b, q_len, n, h).transpose(0, 2, 1, 3)
    k = (key @ Wk + bk).reshape(b, k_len, n, h).transpose(0, 2, 1, 3)
    v = (value @ Wv + bv).reshape(b, k_len, n, h).transpose(0, 2, 1, 3)
    s = np.einsum('bnqh,bnkh->bnqk', q, k) / np.sqrt(h).astype(np.float32)
    s = s + mask
    s = s - s.max(-1, keepdims=True)
    w = np.exp(s)
    w /= w.sum(-1, keepdims=True)
    c = np.einsum('bnqk,bnkh->bqnh', w, v)
    return c.reshape(b * q_len, n * h).astype(np.float32)


def kernel(query, key, value, mask, Wq, bq, Wk, bk, Wv, bv,
           batch_size=B, num_heads=N_HEADS, _trace=False, _trace_kwargs=None):
    in_maps, has_bias, has_mask = shard_inputs(
        query, key, value, mask, Wq, bq, Wk, bk, Wv, bv, batch_size, num_heads)
    if has_bias or has_mask:
        # not exercised by this problem's inputs (zeros); keep a correct path
        return _reference_fallback(query, key, value, mask, Wq, bq, Wk, bk,
                                   Wv, bv, batch_size, num_heads)
    nc = _get_nc()
    res = run_bass_kernel_spmd(nc, in_maps, list(range(8)), trace=_trace,
                               **(_trace_kwargs or {}))
    full = assemble(res.results)
    if _trace:
        return full, res
    return full
